# revision 24
# baseline (speedup 1.0000x reference)
"""GQA attention kernel for 8 Trainium2 NeuronCores.

Sharding: core = (batch b, kv_group g), b in {0,1}, g in {0..3}.
Each core computes the 4 heads of one KV group for one batch and the
partial output projection for those heads; the host sums the 4 group
partials per batch.  Zero duplicated compute across cores.

Fully fused single-pipeline design (v2):
  - every matmul operand is bf16: FWL + background weight buffer hide
    LDWEIGHTS, DMA and SBUF halve; PSUM accumulation stays fp32.
    Measured end-to-end max rel err ~6e-3 vs the 2e-2 gate.
  - one flat instruction stream: K/V/Q(h0) projections run up front,
    the remaining Q projections and the whole output projection are
    emitted as "side units" interleaved between attention tiles, so
    the PE never idles at phase boundaries and the ~55us output
    projection largely hides under the ACT/DVE-bound attention loop.
  - softmax sums off the PE: probs tiles accumulate on DVE in bf16
    (two independent 8-deep chains), then one tiny 4-matmul
    partition-reduce; normalization is a single GpSimd divide.
  - per-e-tile-grouped DMAs ordered so the first projection matmul
    starts as soon as ~0.6 MiB has landed.
"""

import numpy as np

# problem shape (hardcoded per contract)
B, S, E = 2, 2048, 2048
H, G, D = 16, 4, 128
R = H // G          # heads per kv group = 4
ST = S // 128       # 16 t-tiles
ET = E // 128       # 16 e-tiles
SC = S // 512       # 4 s-chunks
NPAIR = S // 1024   # 2 q-chunk pairs

_cache = {}


def _split_multi_waits(nc, maxw=1):
    """Walrus in this container accepts only one sync-wait per
    instruction; move extra waits onto preceding same-engine NoOps."""
    from concourse import mybir

    n_split = 0
    for fn in nc.m.functions:
        for bb in fn.blocks:
            out = []
            changed = False
            for inst in bb.instructions:
                si = inst.sync_info
                waits = list(si.on_wait or []) if si is not None else []
                if len(waits) > maxw:
                    changed = True
                    n_split += 1
                    head, tail = waits[:-maxw], waits[-maxw:]
                    for j in range(0, len(head), maxw):
                        nop = mybir.InstNoOp(
                            name=f"{inst.name}-wsplit{j}", ins=[], outs=[]
                        )
                        nop.engine = inst.engine
                        nop.sync_info = mybir.SyncInfo(
                            on_wait=head[j : j + maxw], on_update=[]
                        )
                        out.append(nop)
                    si.on_wait = tail
                out.append(inst)
            if changed:
                bb.instructions = out
    return n_split


def _build_program():
    import contextlib

    import concourse.bass as bass
    import concourse.tile as tile
    from concourse import mybir
    from concourse.masks import make_identity

    BF16 = mybir.dt.bfloat16
    F32 = mybir.dt.float32
    Exp = mybir.ActivationFunctionType.Exp
    Add = mybir.AluOpType.add
    Mult = mybir.AluOpType.mult

    nc = bass.Bass(target_bir_lowering=False)

    xT = nc.dram_tensor("xT", [E, S], BF16, kind="ExternalInput")
    wq = nc.dram_tensor("wq", [E, R * D], BF16, kind="ExternalInput")
    wk = nc.dram_tensor("wk", [E, D], BF16, kind="ExternalInput")
    wv = nc.dram_tensor("wv", [E, D], BF16, kind="ExternalInput")
    wo = nc.dram_tensor("wo", [R * D, E], BF16, kind="ExternalInput")
    bqv = nc.dram_tensor("bqv", [R * D], F32, kind="ExternalInput")
    bkv = nc.dram_tensor("bkv", [D], F32, kind="ExternalInput")
    bvv = nc.dram_tensor("bvv", [D], F32, kind="ExternalInput")
    otd = nc.dram_tensor("ot", [E, S], BF16, kind="ExternalOutput")

    with tile.TileContext(nc) as tc:
        with contextlib.ExitStack() as ctx:
            ep = ctx.enter_context
            consts = ep(tc.tile_pool(name="consts", bufs=1))
            main = ep(tc.tile_pool(name="main", bufs=1))
            probs_pool = ep(tc.tile_pool(name="probs", bufs=3))
            accp = ep(tc.tile_pool(name="accp", bufs=2))
            normp = ep(tc.tile_pool(name="normp", bufs=2))
            ostage = ep(tc.tile_pool(name="ostage", bufs=3))
            psP = ep(tc.tile_pool(name="psP", bufs=2, space="PSUM"))
            psS = ep(tc.tile_pool(name="psS", bufs=2, space="PSUM"))
            psA = ep(tc.tile_pool(name="psA", bufs=1, space="PSUM"))

            ident_f = consts.tile([128, 128], F32)
            make_identity(nc, ident_f)
            ident = consts.tile([128, 128], BF16)
            nc.vector.tensor_copy(ident, ident_f)
            ones_f = consts.tile([128, 128], F32)
            nc.gpsimd.memset(ones_f, 1.0)
            ones = consts.tile([128, 128], BF16)
            nc.vector.tensor_copy(ones, ones_f)
            bq_sb = consts.tile([128, R], F32)
            nc.sync.dma_start(bq_sb, bqv.rearrange("(o p) -> p o", p=128))
            bk_sb = consts.tile([128, 1], F32)
            nc.sync.dma_start(bk_sb, bkv.rearrange("(o p) -> p o", p=128))
            bv_sb = consts.tile([128, 1], F32)
            nc.sync.dma_start(bv_sb, bvv.rearrange("(o p) -> p o", p=128))

            QT = main.tile([128, R, S], BF16)    # QT[d, h, s]
            KT = main.tile([128, S], BF16)       # KT[d, t]
            V = main.tile([128, ST, D], BF16)    # V[t%128, tt, d]
            VT = main.tile([128, S], BF16)
            outT = main.tile([128, R, S], BF16)  # normalized attn out
            wk_sb = main.tile([128, ET, D], BF16)
            wv_sb = main.tile([128, ET, D], BF16)
            wq_sb = main.tile([128, ET, R * D], BF16)
            wo_sb = main.tile([128, R, E], BF16)
            xtiles = [
                main.tile([128, ET, 512], BF16, name=f"xtile{sc}")
                for sc in range(SC)
            ]

            # DMA trigger order is the startup critical path: K weights and
            # the first x chunks go first so the PE starts ~7us in.
            xTr = xT.rearrange("(o p) m -> p o m", p=128)
            wqr = wq.rearrange("(o p) m -> p o m", p=128)
            nc.sync.dma_start(wk_sb, wk.rearrange("(o p) m -> p o m", p=128))

            def dma_x(sc, eng):
                csx = slice(sc * 512, (sc + 1) * 512)
                for e4 in range(0, ET, 4):
                    eng.dma_start(
                        xtiles[sc][:, e4 : e4 + 4], xTr[:, e4 : e4 + 4, csx]
                    )

            # descriptor generation is ~1.4us per trigger and serializes
            # per-queue: spread the input loads over three engine queues
            dma_x(0, nc.scalar)
            nc.sync.dma_start(wv_sb, wv.rearrange("(o p) m -> p o m", p=128))
            dma_x(1, nc.gpsimd)
            for h in range(R):
                nc.sync.dma_start(
                    wq_sb[:, :, h * 128 : (h + 1) * 128],
                    wqr[:, :, h * 128 : (h + 1) * 128],
                )
            dma_x(2, nc.scalar)
            dma_x(3, nc.gpsimd)
            nc.sync.dma_start(wo_sb, wo.rearrange("(h p) m -> p h m", p=128))

            # ---------- work units ----------
            def unit_proj(kind, sc, h=0):
                cs = slice(sc * 512, (sc + 1) * 512)
                psum = psP.tile([128, 512], F32, tag="p1", name="psum")
                for e in range(ET):
                    if kind == "k":
                        lhsT = wk_sb[:, e]
                    elif kind == "v":
                        lhsT = wv_sb[:, e]
                    else:
                        lhsT = wq_sb[:, e, h * 128 : (h + 1) * 128]
                    nc.tensor.matmul(
                        psum, lhsT, xtiles[sc][:, e],
                        start=(e == 0), stop=(e == ET - 1),
                    )
                if kind == "k":
                    nc.scalar.add(KT[:, cs], psum, bk_sb[:, 0:1])
                elif kind == "v":
                    nc.scalar.add(VT[:, cs], psum, bv_sb[:, 0:1])
                    for q in range(4):
                        tt = sc * 4 + q
                        psv = psP.tile([128, 128], BF16, tag="p1", name="psv")
                        nc.tensor.transpose(
                            psv, VT[:, tt * 128 : (tt + 1) * 128], ident
                        )
                        nc.vector.tensor_copy(V[:, tt], psv)
                else:
                    nc.scalar.add(QT[:, h, cs], psum, bq_sb[:, h : h + 1])

            def unit_p3(et, sc):
                ps = psP.tile([128, 512], F32, tag="p1", name="ps3")
                for h in range(R):
                    nc.tensor.matmul(
                        ps,
                        wo_sb[:, h, et * 128 : (et + 1) * 128],
                        outT[:, h, sc * 512 : (sc + 1) * 512],
                        start=(h == 0), stop=(h == R - 1),
                    )
                st = ostage.tile([128, 512], BF16, tag="ost", name="st")
                nc.vector.tensor_copy(st, ps)
                nc.gpsimd.dma_start(
                    otd[et * 128 : (et + 1) * 128, sc * 512 : (sc + 1) * 512],
                    st,
                )

            side = []

            def pump(n):
                for _ in range(n):
                    if side:
                        side.pop(0)()

            # ---------- upfront projections ----------
            for sc in range(SC):
                unit_proj("k", sc)
                unit_proj("v", sc)
            unit_proj("q", 0, 0)
            unit_proj("q", 1, 0)

            # remaining Q projections stream in as side work, ordered by
            # when the attention iterations consume them:
            # pr0 iters need (h,0),(h,1); pr1 iters need (h,2),(h,3)
            for h in range(1, R):
                side.append(lambda h=h: unit_proj("q", 0, h))
                side.append(lambda h=h: unit_proj("q", 1, h))
            for h in range(R):
                side.append(lambda h=h: unit_proj("q", 2, h))
                side.append(lambda h=h: unit_proj("q", 3, h))

            # ---------- attention + interleaved side units ----------
            def mm_scores(pss, h, q0, tt):
                kslice = KT[:, tt * 128 : (tt + 1) * 128]
                for hf in range(2):
                    nc.tensor.matmul(
                        pss[:, hf * 512 : (hf + 1) * 512],
                        kslice,
                        QT[:, h, q0 + hf * 512 : q0 + (hf + 1) * 512],
                        start=True, stop=True,
                    )

            iters = [(pr, h) for pr in range(NPAIR) for h in range(R)]
            deferred = []  # (slot, closure): recip/mult of the PREVIOUS iter

            def flush_deferred(slot):
                while deferred and deferred[0][0] <= slot:
                    deferred.pop(0)[1]()

            for it, (pr, h) in enumerate(iters):
                q0 = pr * 1024
                out_ps = psA.tile([128, 1024], F32, tag="av", name="out_ps")
                pss_tiles = [None, None]
                pss_tiles[0] = psS.tile([128, 1024], F32, tag="sc", name="pss")
                mm_scores(pss_tiles[0], h, q0, 0)
                acc_a = accp.tile([128, 1024], BF16, tag="acca", name="acc_a")
                acc_b = accp.tile([128, 1024], BF16, tag="accb", name="acc_b")
                for tt in range(ST):
                    pt = probs_pool.tile([128, 1024], BF16, tag="pb", name="pt")
                    nc.scalar.activation(pt, pss_tiles[tt % 2], Exp)
                    # keep independent PE work queued ahead of the
                    # exp-gated AV matmuls
                    if tt + 1 < ST:
                        pss_tiles[(tt + 1) % 2] = psS.tile(
                            [128, 1024], F32, tag="sc", name="pss"
                        )
                        mm_scores(pss_tiles[(tt + 1) % 2], h, q0, tt + 1)
                    for hf in range(2):
                        hs = slice(hf * 512, (hf + 1) * 512)
                        nc.tensor.matmul(
                            out_ps[:, hs], V[:, tt], pt[:, hs],
                            start=(tt == 0), stop=(tt == ST - 1),
                        )
                    # softmax denominators: bf16 elementwise accumulation
                    # on DVE (two 8-deep chains), off the PE entirely
                    if tt == 0:
                        nc.vector.tensor_copy(acc_a, pt)
                    elif tt == 1:
                        nc.vector.tensor_copy(acc_b, pt)
                    elif tt % 2 == 0:
                        nc.vector.tensor_tensor(acc_a, acc_a, pt, Add)
                    else:
                        nc.vector.tensor_tensor(acc_b, acc_b, pt, Add)
                    # previous iter's slow reciprocal runs HERE, mid-iter,
                    # where the in-order DVE queue has slack - never at an
                    # iteration boundary where it would gate probs recycling
                    flush_deferred(tt)
                    if tt == 7 or tt == 15 or (it >= 4 and tt in (3, 11)):
                        pump(1)
                # partition-reduce the two chain accumulators: 4 small
                # matmuls -> sums replicated across partitions
                sums_ps = psS.tile([128, 1024], F32, tag="sc", name="sums_ps")
                for ai, acc in enumerate((acc_a, acc_b)):
                    for hf in range(2):
                        hs = slice(hf * 512, (hf + 1) * 512)
                        nc.tensor.matmul(
                            sums_ps[:, hs], ones, acc[:, hs],
                            start=(ai == 0), stop=(ai == 1),
                        )
                sums_sb = normp.tile([128, 1024], F32, tag="s", name="sums_sb")
                nc.vector.tensor_copy(sums_sb, sums_ps)
                av_sb = normp.tile([128, 1024], BF16, tag="a", name="av_sb")
                nc.vector.tensor_copy(av_sb, out_ps)

                def normalize(h=h, q0=q0, sums_sb=sums_sb, av_sb=av_sb):
                    rc = normp.tile([128, 1024], F32, tag="r", name="rc")
                    nc.vector.reciprocal(rc, sums_sb)
                    nc.vector.tensor_tensor(
                        outT[:, h, q0 : q0 + 1024], av_sb, rc, Mult
                    )

                deferred.append((4, normalize))
                if it == 3:
                    # pr0 fully normalized soon: its output projection
                    # columns become available side work
                    for et in range(ET):
                        for sc in range(2):
                            side.append(lambda et=et, sc=sc: unit_p3(et, sc))
            flush_deferred(ST)

            for et in range(ET):
                for sc in range(2, 4):
                    side.append(lambda et=et, sc=sc: unit_p3(et, sc))
            pump(len(side))

    _split_multi_waits(nc)
    return nc


def _prepare(x, Wq, bq, Wk, bk, Wv, bv, Wo, bo):
    """Host-side sharding: build per-core input maps (bf16 operands)."""
    import ml_dtypes

    bf16 = ml_dtypes.bfloat16
    x = np.asarray(x, dtype=np.float32)
    Wq = np.asarray(Wq, dtype=np.float32)
    bq = np.asarray(bq, dtype=np.float32)
    Wk = np.asarray(Wk, dtype=np.float32)
    bk = np.asarray(bk, dtype=np.float32)
    Wv = np.asarray(Wv, dtype=np.float32)
    bv = np.asarray(bv, dtype=np.float32)
    Wo = np.asarray(Wo, dtype=np.float32)

    isd = np.float32(1.0 / np.sqrt(D))
    xTs = [np.ascontiguousarray(x[b].T).astype(bf16) for b in range(B)]
    Wq_s = (Wq * isd).astype(bf16)
    Wk_s = Wk.astype(bf16)
    Wv_s = Wv.astype(bf16)
    Wo_s = Wo.astype(bf16)
    in_maps = []
    for core in range(8):
        b, g = divmod(core, G)
        in_maps.append({
            "xT": xTs[b],
            "wq": np.ascontiguousarray(Wq_s[:, g * R * D : (g + 1) * R * D]),
            "wk": np.ascontiguousarray(Wk_s[:, g * D : (g + 1) * D]),
            "wv": np.ascontiguousarray(Wv_s[:, g * D : (g + 1) * D]),
            "wo": np.ascontiguousarray(Wo_s[g * R * D : (g + 1) * R * D, :]),
            "bqv": bq[g * R * D : (g + 1) * R * D] * isd,
            "bkv": bk[g * D : (g + 1) * D],
            "bvv": bv[g * D : (g + 1) * D],
        })
    return in_maps


def _gather(results, bo):
    bo = np.asarray(bo, dtype=np.float32)
    out = np.empty((B, S, E), dtype=np.float32)
    for b in range(B):
        acc = results[b * G]["ot"].astype(np.float32)
        for g in range(1, G):
            acc += results[b * G + g]["ot"].astype(np.float32)
        out[b] = acc.T + bo
    return out


def kernel(x, Wq, bq, Wk, bk, Wv, bv, Wo, bo):
    from concourse.bass_utils import run_bass_kernel_spmd

    if "nc" not in _cache:
        _cache["nc"] = _build_program()
    nc = _cache["nc"]
    in_maps = _prepare(x, Wq, bq, Wk, bk, Wv, bv, Wo, bo)
    res = run_bass_kernel_spmd(nc, in_maps, core_ids=list(range(8)))
    return _gather(res.results, bo)


# revision 26
# speedup vs baseline: 1.0212x; 1.0212x over previous
"""GQA attention kernel for 8 Trainium2 NeuronCores.

Sharding: core = (batch b, kv_group g), b in {0,1}, g in {0..3}.
Each core computes the 4 heads of one KV group for one batch and the
partial output projection for those heads; the host sums the 4 group
partials per batch.  Zero duplicated compute across cores.

Fully fused single-pipeline design (v2):
  - every matmul operand is bf16: FWL + background weight buffer hide
    LDWEIGHTS, DMA and SBUF halve; PSUM accumulation stays fp32.
    Measured end-to-end max rel err ~6e-3 vs the 2e-2 gate.
  - one flat instruction stream: K/V/Q(h0) projections run up front,
    the remaining Q projections and the whole output projection are
    emitted as "side units" interleaved between attention tiles, so
    the PE never idles at phase boundaries and the ~55us output
    projection largely hides under the ACT/DVE-bound attention loop.
  - softmax sums off the PE: probs tiles accumulate on DVE in bf16
    (two independent 8-deep chains), then one tiny 4-matmul
    partition-reduce; normalization is a single GpSimd divide.
  - per-e-tile-grouped DMAs ordered so the first projection matmul
    starts as soon as ~0.6 MiB has landed.
"""

import numpy as np

# problem shape (hardcoded per contract)
B, S, E = 2, 2048, 2048
H, G, D = 16, 4, 128
R = H // G          # heads per kv group = 4
ST = S // 128       # 16 t-tiles
ET = E // 128       # 16 e-tiles
SC = S // 512       # 4 s-chunks
NPAIR = S // 1024   # 2 q-chunk pairs

_cache = {}


def _split_multi_waits(nc, maxw=1):
    """Walrus in this container accepts only one sync-wait per
    instruction; move extra waits onto preceding same-engine NoOps."""
    from concourse import mybir

    n_split = 0
    for fn in nc.m.functions:
        for bb in fn.blocks:
            out = []
            changed = False
            for inst in bb.instructions:
                si = inst.sync_info
                waits = list(si.on_wait or []) if si is not None else []
                if len(waits) > maxw:
                    changed = True
                    n_split += 1
                    head, tail = waits[:-maxw], waits[-maxw:]
                    for j in range(0, len(head), maxw):
                        nop = mybir.InstNoOp(
                            name=f"{inst.name}-wsplit{j}", ins=[], outs=[]
                        )
                        nop.engine = inst.engine
                        nop.sync_info = mybir.SyncInfo(
                            on_wait=head[j : j + maxw], on_update=[]
                        )
                        out.append(nop)
                    si.on_wait = tail
                out.append(inst)
            if changed:
                bb.instructions = out
    return n_split


def _build_program():
    import contextlib

    import concourse.bass as bass
    import concourse.tile as tile
    from concourse import mybir
    from concourse.masks import make_identity

    BF16 = mybir.dt.bfloat16
    F32 = mybir.dt.float32
    Exp = mybir.ActivationFunctionType.Exp
    Add = mybir.AluOpType.add
    Mult = mybir.AluOpType.mult

    nc = bass.Bass(target_bir_lowering=False)

    xT = nc.dram_tensor("xT", [E, S], BF16, kind="ExternalInput")
    wq = nc.dram_tensor("wq", [E, R * D], BF16, kind="ExternalInput")
    wk = nc.dram_tensor("wk", [E, D], BF16, kind="ExternalInput")
    wv = nc.dram_tensor("wv", [E, D], BF16, kind="ExternalInput")
    wo = nc.dram_tensor("wo", [R * D, E], BF16, kind="ExternalInput")
    bqv = nc.dram_tensor("bqv", [R * D], F32, kind="ExternalInput")
    bkv = nc.dram_tensor("bkv", [D], F32, kind="ExternalInput")
    bvv = nc.dram_tensor("bvv", [D], F32, kind="ExternalInput")
    otd = nc.dram_tensor("ot", [E, S], BF16, kind="ExternalOutput")

    with tile.TileContext(nc) as tc:
        with contextlib.ExitStack() as ctx:
            ep = ctx.enter_context
            consts = ep(tc.tile_pool(name="consts", bufs=1))
            main = ep(tc.tile_pool(name="main", bufs=1))
            probs_pool = ep(tc.tile_pool(name="probs", bufs=3))
            accp = ep(tc.tile_pool(name="accp", bufs=2))
            normp = ep(tc.tile_pool(name="normp", bufs=2))
            ostage = ep(tc.tile_pool(name="ostage", bufs=3))
            psP = ep(tc.tile_pool(name="psP", bufs=2, space="PSUM"))
            psS = ep(tc.tile_pool(name="psS", bufs=2, space="PSUM"))
            psA = ep(tc.tile_pool(name="psA", bufs=1, space="PSUM"))

            ident_f = consts.tile([128, 128], F32)
            make_identity(nc, ident_f)
            ident = consts.tile([128, 128], BF16)
            nc.vector.tensor_copy(ident, ident_f)
            ones_f = consts.tile([128, 128], F32)
            nc.gpsimd.memset(ones_f, 1.0)
            ones = consts.tile([128, 128], BF16)
            nc.vector.tensor_copy(ones, ones_f)
            bq_sb = consts.tile([128, R], F32)
            nc.sync.dma_start(bq_sb, bqv.rearrange("(o p) -> p o", p=128))
            bk_sb = consts.tile([128, 1], F32)
            nc.sync.dma_start(bk_sb, bkv.rearrange("(o p) -> p o", p=128))
            bv_sb = consts.tile([128, 1], F32)
            nc.sync.dma_start(bv_sb, bvv.rearrange("(o p) -> p o", p=128))

            QT = main.tile([128, R, S], BF16)    # QT[d, h, s]
            KT = main.tile([128, S], BF16)       # KT[d, t]
            V = main.tile([128, ST, D], BF16)    # V[t%128, tt, d]
            VT = main.tile([128, S], BF16)
            outT = main.tile([128, R, S], BF16)  # normalized attn out
            wk_sb = main.tile([128, ET, D], BF16)
            wv_sb = main.tile([128, ET, D], BF16)
            wq_sb = main.tile([128, ET, R * D], BF16)
            wo_sb = main.tile([128, R, E], BF16)
            xtiles = [
                main.tile([128, ET, 512], BF16, name=f"xtile{sc}")
                for sc in range(SC)
            ]

            # DMA trigger order is the startup critical path: K weights and
            # the first x chunks go first so the PE starts ~7us in.
            xTr = xT.rearrange("(o p) m -> p o m", p=128)
            wqr = wq.rearrange("(o p) m -> p o m", p=128)
            nc.sync.dma_start(wk_sb, wk.rearrange("(o p) m -> p o m", p=128))

            def dma_x(sc, eng):
                csx = slice(sc * 512, (sc + 1) * 512)
                for e4 in range(0, ET, 4):
                    eng.dma_start(
                        xtiles[sc][:, e4 : e4 + 4], xTr[:, e4 : e4 + 4, csx]
                    )

            # descriptor generation is ~1.4us per trigger and serializes
            # per-queue: spread the input loads over three engine queues
            dma_x(0, nc.sync)
            nc.sync.dma_start(wv_sb, wv.rearrange("(o p) m -> p o m", p=128))
            dma_x(1, nc.sync)
            for h in range(R):
                nc.sync.dma_start(
                    wq_sb[:, :, h * 128 : (h + 1) * 128],
                    wqr[:, :, h * 128 : (h + 1) * 128],
                )
            dma_x(2, nc.scalar)
            dma_x(3, nc.gpsimd)
            nc.sync.dma_start(wo_sb, wo.rearrange("(h p) m -> p h m", p=128))

            # ---------- work units ----------
            def unit_proj(kind, sc, h=0):
                cs = slice(sc * 512, (sc + 1) * 512)
                psum = psP.tile([128, 512], F32, tag="p1", name="psum")
                for e in range(ET):
                    if kind == "k":
                        lhsT = wk_sb[:, e]
                    elif kind == "v":
                        lhsT = wv_sb[:, e]
                    else:
                        lhsT = wq_sb[:, e, h * 128 : (h + 1) * 128]
                    nc.tensor.matmul(
                        psum, lhsT, xtiles[sc][:, e],
                        start=(e == 0), stop=(e == ET - 1),
                    )
                if kind == "k":
                    nc.scalar.add(KT[:, cs], psum, bk_sb[:, 0:1])
                elif kind == "v":
                    nc.scalar.add(VT[:, cs], psum, bv_sb[:, 0:1])
                    for q in range(4):
                        tt = sc * 4 + q
                        psv = psP.tile([128, 128], BF16, tag="p1", name="psv")
                        nc.tensor.transpose(
                            psv, VT[:, tt * 128 : (tt + 1) * 128], ident
                        )
                        nc.vector.tensor_copy(V[:, tt], psv)
                else:
                    nc.scalar.add(QT[:, h, cs], psum, bq_sb[:, h : h + 1])

            def unit_p3(et, sc):
                ps = psP.tile([128, 512], F32, tag="p1", name="ps3")
                for h in range(R):
                    nc.tensor.matmul(
                        ps,
                        wo_sb[:, h, et * 128 : (et + 1) * 128],
                        outT[:, h, sc * 512 : (sc + 1) * 512],
                        start=(h == 0), stop=(h == R - 1),
                    )
                st = ostage.tile([128, 512], BF16, tag="ost", name="st")
                nc.vector.tensor_copy(st, ps)
                nc.gpsimd.dma_start(
                    otd[et * 128 : (et + 1) * 128, sc * 512 : (sc + 1) * 512],
                    st,
                )

            side = []

            def pump(n):
                for _ in range(n):
                    if side:
                        side.pop(0)()

            # ---------- upfront projections ----------
            for sc in range(SC):
                unit_proj("k", sc)
                unit_proj("v", sc)
            unit_proj("q", 0, 0)
            unit_proj("q", 1, 0)

            # remaining Q projections stream in as side work, ordered by
            # when the attention iterations consume them:
            # pr0 iters need (h,0),(h,1); pr1 iters need (h,2),(h,3)
            for h in range(1, R):
                side.append(lambda h=h: unit_proj("q", 0, h))
                side.append(lambda h=h: unit_proj("q", 1, h))
            for h in range(R):
                side.append(lambda h=h: unit_proj("q", 2, h))
                side.append(lambda h=h: unit_proj("q", 3, h))

            # ---------- attention + interleaved side units ----------
            def mm_scores(pss, h, q0, tt):
                kslice = KT[:, tt * 128 : (tt + 1) * 128]
                for hf in range(2):
                    nc.tensor.matmul(
                        pss[:, hf * 512 : (hf + 1) * 512],
                        kslice,
                        QT[:, h, q0 + hf * 512 : q0 + (hf + 1) * 512],
                        start=True, stop=True,
                    )

            iters = [(pr, h) for pr in range(NPAIR) for h in range(R)]
            deferred = []  # (slot, closure): recip/mult of the PREVIOUS iter

            def flush_deferred(slot):
                while deferred and deferred[0][0] <= slot:
                    deferred.pop(0)[1]()

            for it, (pr, h) in enumerate(iters):
                q0 = pr * 1024
                out_ps = psA.tile([128, 1024], F32, tag="av", name="out_ps")
                pss_tiles = [None, None]
                pss_tiles[0] = psS.tile([128, 1024], F32, tag="sc", name="pss")
                mm_scores(pss_tiles[0], h, q0, 0)
                acc_a = accp.tile([128, 1024], BF16, tag="acca", name="acc_a")
                acc_b = accp.tile([128, 1024], BF16, tag="accb", name="acc_b")
                for tt in range(ST):
                    pt = probs_pool.tile([128, 1024], BF16, tag="pb", name="pt")
                    nc.scalar.activation(pt, pss_tiles[tt % 2], Exp)
                    # keep independent PE work queued ahead of the
                    # exp-gated AV matmuls
                    if tt + 1 < ST:
                        pss_tiles[(tt + 1) % 2] = psS.tile(
                            [128, 1024], F32, tag="sc", name="pss"
                        )
                        mm_scores(pss_tiles[(tt + 1) % 2], h, q0, tt + 1)
                    for hf in range(2):
                        hs = slice(hf * 512, (hf + 1) * 512)
                        nc.tensor.matmul(
                            out_ps[:, hs], V[:, tt], pt[:, hs],
                            start=(tt == 0), stop=(tt == ST - 1),
                        )
                    # softmax denominators: bf16 elementwise accumulation
                    # on DVE (two 8-deep chains), off the PE entirely
                    if tt == 0:
                        nc.vector.tensor_copy(acc_a, pt)
                    elif tt == 1:
                        nc.vector.tensor_copy(acc_b, pt)
                    elif tt % 2 == 0:
                        nc.vector.tensor_tensor(acc_a, acc_a, pt, Add)
                    else:
                        nc.vector.tensor_tensor(acc_b, acc_b, pt, Add)
                    # previous iter's slow reciprocal runs HERE, mid-iter,
                    # where the in-order DVE queue has slack - never at an
                    # iteration boundary where it would gate probs recycling
                    flush_deferred(tt)
                    if tt == 7 or tt == 15 or (it >= 4 and tt in (3, 11)):
                        pump(1)
                # partition-reduce the two chain accumulators: 4 small
                # matmuls -> sums replicated across partitions
                sums_ps = psS.tile([128, 1024], F32, tag="sc", name="sums_ps")
                for ai, acc in enumerate((acc_a, acc_b)):
                    for hf in range(2):
                        hs = slice(hf * 512, (hf + 1) * 512)
                        nc.tensor.matmul(
                            sums_ps[:, hs], ones, acc[:, hs],
                            start=(ai == 0), stop=(ai == 1),
                        )
                sums_sb = normp.tile([128, 1024], F32, tag="s", name="sums_sb")
                nc.vector.tensor_copy(sums_sb, sums_ps)
                av_sb = normp.tile([128, 1024], BF16, tag="a", name="av_sb")
                nc.vector.tensor_copy(av_sb, out_ps)

                # the ~6.5us DVE reciprocal would starve probs recycling if
                # run whole: split into 4 chunks spread between next-iter
                # adds, with the normalize multiply trailing
                rc = normp.tile([128, 1024], F32, tag="r", name="rc")

                def rchunk(c, sums_sb=sums_sb, rc=rc):
                    cs4 = slice(c * 256, (c + 1) * 256)
                    nc.vector.reciprocal(rc[:, cs4], sums_sb[:, cs4])

                def fmult(h=h, q0=q0, av_sb=av_sb, rc=rc):
                    nc.vector.tensor_tensor(
                        outT[:, h, q0 : q0 + 1024], av_sb, rc, Mult
                    )

                for c in range(4):
                    deferred.append((3 + 2 * c, lambda c=c: rchunk(c)))
                deferred.append((11, fmult))
                if it == 3:
                    # pr0 fully normalized soon: its output projection
                    # columns become available side work
                    for et in range(ET):
                        for sc in range(2):
                            side.append(lambda et=et, sc=sc: unit_p3(et, sc))
            flush_deferred(ST)

            for et in range(ET):
                for sc in range(2, 4):
                    side.append(lambda et=et, sc=sc: unit_p3(et, sc))
            pump(len(side))

    _split_multi_waits(nc)
    return nc


def _prepare(x, Wq, bq, Wk, bk, Wv, bv, Wo, bo):
    """Host-side sharding: build per-core input maps (bf16 operands)."""
    import ml_dtypes

    bf16 = ml_dtypes.bfloat16
    x = np.asarray(x, dtype=np.float32)
    Wq = np.asarray(Wq, dtype=np.float32)
    bq = np.asarray(bq, dtype=np.float32)
    Wk = np.asarray(Wk, dtype=np.float32)
    bk = np.asarray(bk, dtype=np.float32)
    Wv = np.asarray(Wv, dtype=np.float32)
    bv = np.asarray(bv, dtype=np.float32)
    Wo = np.asarray(Wo, dtype=np.float32)

    isd = np.float32(1.0 / np.sqrt(D))
    xTs = [np.ascontiguousarray(x[b].T).astype(bf16) for b in range(B)]
    Wq_s = (Wq * isd).astype(bf16)
    Wk_s = Wk.astype(bf16)
    Wv_s = Wv.astype(bf16)
    Wo_s = Wo.astype(bf16)
    in_maps = []
    for core in range(8):
        b, g = divmod(core, G)
        in_maps.append({
            "xT": xTs[b],
            "wq": np.ascontiguousarray(Wq_s[:, g * R * D : (g + 1) * R * D]),
            "wk": np.ascontiguousarray(Wk_s[:, g * D : (g + 1) * D]),
            "wv": np.ascontiguousarray(Wv_s[:, g * D : (g + 1) * D]),
            "wo": np.ascontiguousarray(Wo_s[g * R * D : (g + 1) * R * D, :]),
            "bqv": bq[g * R * D : (g + 1) * R * D] * isd,
            "bkv": bk[g * D : (g + 1) * D],
            "bvv": bv[g * D : (g + 1) * D],
        })
    return in_maps


def _gather(results, bo):
    bo = np.asarray(bo, dtype=np.float32)
    out = np.empty((B, S, E), dtype=np.float32)
    for b in range(B):
        acc = results[b * G]["ot"].astype(np.float32)
        for g in range(1, G):
            acc += results[b * G + g]["ot"].astype(np.float32)
        out[b] = acc.T + bo
    return out


def kernel(x, Wq, bq, Wk, bk, Wv, bv, Wo, bo):
    from concourse.bass_utils import run_bass_kernel_spmd

    if "nc" not in _cache:
        _cache["nc"] = _build_program()
    nc = _cache["nc"]
    in_maps = _prepare(x, Wq, bq, Wk, bk, Wv, bv, Wo, bo)
    res = run_bass_kernel_spmd(nc, in_maps, core_ids=list(range(8)))
    return _gather(res.results, bo)


# revision 38
# speedup vs baseline: 1.1325x; 1.1090x over previous
"""GQA attention kernel for 8 Trainium2 NeuronCores.

Sharding: core = (batch b, kv_group g), b in {0,1}, g in {0..3}.
Each core computes the 4 heads of one KV group for one batch and the
partial output projection for those heads; the host sums the 4 group
partials per batch.  Zero duplicated compute across cores.

Fully fused single-pipeline design (v2):
  - every matmul operand is bf16: FWL + background weight buffer hide
    LDWEIGHTS, DMA and SBUF halve; PSUM accumulation stays fp32.
    Measured end-to-end max rel err ~6e-3 vs the 2e-2 gate.
  - one flat instruction stream: K/V/Q(h0) projections run up front,
    the remaining Q projections and the whole output projection are
    emitted as "side units" interleaved between attention tiles, so
    the PE never idles at phase boundaries and the ~55us output
    projection largely hides under the ACT/DVE-bound attention loop.
  - softmax sums off the PE: probs tiles accumulate on DVE in bf16
    (two independent 8-deep chains), then one tiny 4-matmul
    partition-reduce; normalization is a single GpSimd divide.
  - per-e-tile-grouped DMAs ordered so the first projection matmul
    starts as soon as ~0.6 MiB has landed.
"""

import numpy as np

# problem shape (hardcoded per contract)
B, S, E = 2, 2048, 2048
H, G, D = 16, 4, 128
R = H // G          # heads per kv group = 4
ST = S // 128       # 16 t-tiles
ET = E // 128       # 16 e-tiles
SC = S // 512       # 4 s-chunks
NPAIR = S // 1024   # 2 q-chunk pairs

_cache = {}


def _split_multi_waits(nc, maxw=1):
    """Walrus in this container accepts only one sync-wait per
    instruction; move extra waits onto preceding same-engine NoOps."""
    from concourse import mybir

    n_split = 0
    for fn in nc.m.functions:
        for bb in fn.blocks:
            out = []
            changed = False
            for inst in bb.instructions:
                si = inst.sync_info
                waits = list(si.on_wait or []) if si is not None else []
                if len(waits) > maxw:
                    changed = True
                    n_split += 1
                    head, tail = waits[:-maxw], waits[-maxw:]
                    for j in range(0, len(head), maxw):
                        nop = mybir.InstNoOp(
                            name=f"{inst.name}-wsplit{j}", ins=[], outs=[]
                        )
                        nop.engine = inst.engine
                        nop.sync_info = mybir.SyncInfo(
                            on_wait=head[j : j + maxw], on_update=[]
                        )
                        out.append(nop)
                    si.on_wait = tail
                out.append(inst)
            if changed:
                bb.instructions = out
    return n_split


def _build_program():
    import contextlib

    import concourse.bass as bass
    import concourse.tile as tile
    from concourse import mybir
    from concourse.masks import make_identity

    BF16 = mybir.dt.bfloat16
    F32 = mybir.dt.float32
    Exp = mybir.ActivationFunctionType.Exp
    Add = mybir.AluOpType.add
    Mult = mybir.AluOpType.mult

    nc = bass.Bass(target_bir_lowering=False)

    # inputs arrive pre-tiled by the host into the exact SBUF image
    # (partition-major), so every DMA is long contiguous runs: one
    # trigger, 128 descriptors, full bandwidth
    xd = nc.dram_tensor("xd", [128, SC, ET, 512], BF16, kind="ExternalInput")
    wq = nc.dram_tensor("wq", [128, ET, R * D], BF16, kind="ExternalInput")
    wk = nc.dram_tensor("wk", [128, ET, D], BF16, kind="ExternalInput")
    wv = nc.dram_tensor("wv", [128, ET, D], BF16, kind="ExternalInput")
    wo = nc.dram_tensor("wo", [128, R, E], BF16, kind="ExternalInput")
    bqv = nc.dram_tensor("bqv", [R * D], F32, kind="ExternalInput")
    bkv = nc.dram_tensor("bkv", [D], F32, kind="ExternalInput")
    bvv = nc.dram_tensor("bvv", [D], F32, kind="ExternalInput")
    otd = nc.dram_tensor("ot", [E, S], BF16, kind="ExternalOutput")

    with tile.TileContext(nc) as tc:
        with contextlib.ExitStack() as ctx:
            ep = ctx.enter_context
            consts = ep(tc.tile_pool(name="consts", bufs=1))
            main = ep(tc.tile_pool(name="main", bufs=1))
            probs_pool = ep(tc.tile_pool(name="probs", bufs=6))
            accp = ep(tc.tile_pool(name="accp", bufs=2))
            normp = ep(tc.tile_pool(name="normp", bufs=2))
            ostage = ep(tc.tile_pool(name="ostage", bufs=3))
            psP = ep(tc.tile_pool(name="psP", bufs=2, space="PSUM"))
            psS = ep(tc.tile_pool(name="psS", bufs=2, space="PSUM"))
            psA = ep(tc.tile_pool(name="psA", bufs=1, space="PSUM"))

            ident_f = consts.tile([128, 128], F32)
            make_identity(nc, ident_f)
            ident = consts.tile([128, 128], BF16)
            nc.vector.tensor_copy(ident, ident_f)
            ones_f = consts.tile([128, 128], F32)
            nc.gpsimd.memset(ones_f, 1.0)
            ones = consts.tile([128, 128], BF16)
            nc.vector.tensor_copy(ones, ones_f)
            bq_sb = consts.tile([128, R], F32)
            nc.sync.dma_start(bq_sb, bqv.rearrange("(o p) -> p o", p=128))
            bk_sb = consts.tile([128, 1], F32)
            nc.sync.dma_start(bk_sb, bkv.rearrange("(o p) -> p o", p=128))
            bv_sb = consts.tile([128, 1], F32)
            nc.sync.dma_start(bv_sb, bvv.rearrange("(o p) -> p o", p=128))

            QT = main.tile([128, R, S], BF16)    # QT[d, h, s]
            KT = main.tile([128, S], BF16)       # KT[d, t]
            V = main.tile([128, ST, D], BF16)    # V[t%128, tt, d]
            VT = main.tile([128, S], BF16)
            outT = main.tile([128, R, S], BF16)  # normalized attn out
            wk_sb = main.tile([128, ET, D], BF16)
            wv_sb = main.tile([128, ET, D], BF16)
            wq_sb = main.tile([128, ET, R * D], BF16)
            wo_sb = main.tile([128, R, E], BF16)
            xtiles = [
                main.tile([128, ET, 512], BF16, name=f"xtile{sc}")
                for sc in range(SC)
            ]

            # DMA trigger order is the startup critical path: K weights and
            # the first x chunks go first so the PE starts early; x chunks
            # arrive in the order the upfront K/V units consume them
            def dma_x(sc):
                # halves: lets the first projection matmuls start while the
                # second half of the chunk is still in flight
                nc.sync.dma_start(xtiles[sc][:, 0:8], xd[:, sc, 0:8])
                nc.sync.dma_start(xtiles[sc][:, 8:16], xd[:, sc, 8:16])

            dma_x(0)
            nc.sync.dma_start(wk_sb, wk[:, :, :])
            dma_x(1)
            nc.sync.dma_start(wv_sb, wv[:, :, :])
            dma_x(2)
            dma_x(3)
            nc.sync.dma_start(wq_sb, wq[:, :, :])
            nc.sync.dma_start(wo_sb, wo[:, :, :])

            # ---------- work units ----------
            def unit_proj(kind, sc, h=0):
                cs = slice(sc * 512, (sc + 1) * 512)
                psum = psP.tile([128, 512], F32, tag="p1", name="psum")
                for e in range(ET):
                    if kind == "k":
                        lhsT = wk_sb[:, e]
                    elif kind == "v":
                        lhsT = wv_sb[:, e]
                    else:
                        lhsT = wq_sb[:, e, h * 128 : (h + 1) * 128]
                    nc.tensor.matmul(
                        psum, lhsT, xtiles[sc][:, e],
                        start=(e == 0), stop=(e == ET - 1),
                    )
                if kind == "k":
                    nc.scalar.add(KT[:, cs], psum, bk_sb[:, 0:1])
                elif kind == "v":
                    nc.scalar.add(VT[:, cs], psum, bv_sb[:, 0:1])
                    for q in range(4):
                        tt = sc * 4 + q
                        psv = psP.tile([128, 128], BF16, tag="p1", name="psv")
                        nc.tensor.transpose(
                            psv, VT[:, tt * 128 : (tt + 1) * 128], ident
                        )
                        nc.vector.tensor_copy(V[:, tt], psv)
                else:
                    nc.scalar.add(QT[:, h, cs], psum, bq_sb[:, h : h + 1])

            def unit_p3(et, sc):
                ps = psP.tile([128, 512], F32, tag="p1", name="ps3")
                for h in range(R):
                    nc.tensor.matmul(
                        ps,
                        wo_sb[:, h, et * 128 : (et + 1) * 128],
                        outT[:, h, sc * 512 : (sc + 1) * 512],
                        start=(h == 0), stop=(h == R - 1),
                    )
                st = ostage.tile([128, 512], BF16, tag="ost", name="st")
                nc.vector.tensor_copy(st, ps)
                nc.gpsimd.dma_start(
                    otd[et * 128 : (et + 1) * 128, sc * 512 : (sc + 1) * 512],
                    st,
                )

            side = []

            def pump(n):
                for _ in range(n):
                    if side:
                        side.pop(0)()

            # ---------- upfront projections (interleaved K/V per chunk to
            # match x chunk arrival order) ----------
            for sc in range(SC):
                unit_proj("k", sc)
                unit_proj("v", sc)
            unit_proj("q", 0, 0)
            unit_proj("q", 1, 0)

            # remaining Q projections stream in as side work, ordered by
            # when the attention iterations consume them:
            # pr0 iters need (h,0),(h,1); pr1 iters need (h,2),(h,3)
            for h in range(1, R):
                side.append(lambda h=h: unit_proj("q", 0, h))
                side.append(lambda h=h: unit_proj("q", 1, h))
            for h in range(R):
                side.append(lambda h=h: unit_proj("q", 2, h))
                side.append(lambda h=h: unit_proj("q", 3, h))

            # ---------- attention + interleaved side units ----------
            def mm_scores(pss, h, q0, tt):
                kslice = KT[:, tt * 128 : (tt + 1) * 128]
                for hf in range(2):
                    nc.tensor.matmul(
                        pss[:, hf * 512 : (hf + 1) * 512],
                        kslice,
                        QT[:, h, q0 + hf * 512 : q0 + (hf + 1) * 512],
                        start=True, stop=True,
                    )

            iters = [(pr, h) for pr in range(NPAIR) for h in range(R)]
            deferred = []  # (slot, closure): recip/mult of the PREVIOUS iter

            def flush_deferred(slot):
                while deferred and deferred[0][0] <= slot:
                    deferred.pop(0)[1]()

            for it, (pr, h) in enumerate(iters):
                q0 = pr * 1024
                out_ps = psA.tile([128, 1024], F32, tag="av", name="out_ps")
                pss_tiles = [None, None]
                pss_tiles[0] = psS.tile([128, 1024], F32, tag="sc", name="pss")
                mm_scores(pss_tiles[0], h, q0, 0)
                acc_a = accp.tile([128, 1024], BF16, tag="acca", name="acc_a")
                acc_b = accp.tile([128, 1024], BF16, tag="accb", name="acc_b")
                for tt in range(ST):
                    pt = probs_pool.tile([128, 1024], BF16, tag="pb", name="pt")
                    nc.scalar.activation(pt, pss_tiles[tt % 2], Exp)
                    # keep independent PE work queued ahead of the
                    # exp-gated AV matmuls
                    if tt + 1 < ST:
                        pss_tiles[(tt + 1) % 2] = psS.tile(
                            [128, 1024], F32, tag="sc", name="pss"
                        )
                        mm_scores(pss_tiles[(tt + 1) % 2], h, q0, tt + 1)
                    for hf in range(2):
                        hs = slice(hf * 512, (hf + 1) * 512)
                        nc.tensor.matmul(
                            out_ps[:, hs], V[:, tt], pt[:, hs],
                            start=(tt == 0), stop=(tt == ST - 1),
                        )
                    # softmax denominators: bf16 elementwise accumulation
                    # on DVE (two 8-deep chains), off the PE entirely
                    if tt == 0:
                        nc.vector.tensor_copy(acc_a, pt)
                    elif tt == 1:
                        nc.vector.tensor_copy(acc_b, pt)
                    elif tt % 2 == 0:
                        nc.vector.tensor_tensor(acc_a, acc_a, pt, Add)
                    else:
                        nc.vector.tensor_tensor(acc_b, acc_b, pt, Add)
                    # previous iter's slow reciprocal runs HERE, mid-iter,
                    # where the in-order DVE queue has slack - never at an
                    # iteration boundary where it would gate probs recycling
                    flush_deferred(tt)
                    if tt == 7 or tt == 15 or (it >= 4 and tt in (3, 11)):
                        pump(1)
                # partition-reduce the two chain accumulators: 4 small
                # matmuls -> sums replicated across partitions
                sums_ps = psS.tile([128, 1024], F32, tag="sc", name="sums_ps")
                for ai, acc in enumerate((acc_a, acc_b)):
                    for hf in range(2):
                        hs = slice(hf * 512, (hf + 1) * 512)
                        nc.tensor.matmul(
                            sums_ps[:, hs], ones, acc[:, hs],
                            start=(ai == 0), stop=(ai == 1),
                        )
                av_sb = normp.tile([128, 1024], BF16, tag="a", name="av_sb")
                nc.vector.tensor_copy(av_sb, out_ps)
                sums_sb = normp.tile([128, 1024], F32, tag="s", name="sums_sb")
                nc.vector.tensor_copy(sums_sb, sums_ps)

                # the ~6.5us DVE reciprocal would starve probs recycling if
                # run whole: split into 4 chunks spread between next-iter
                # adds, with the normalize multiply trailing
                rc = normp.tile([128, 1024], BF16, tag="r", name="rc")

                def rchunk(c, sums_sb=sums_sb, rc=rc):
                    cs4 = slice(c * 256, (c + 1) * 256)
                    with nc.allow_low_precision(
                        reason="bf16 softmax scale, ~0.4% ok at 2e-2 gate"
                    ):
                        nc.vector.reciprocal(rc[:, cs4], sums_sb[:, cs4])

                def fmult(h=h, q0=q0, av_sb=av_sb, rc=rc):
                    # all-bf16 multiply on the otherwise-idle GpSimd engine
                    nc.gpsimd.tensor_tensor(
                        outT[:, h, q0 : q0 + 1024], av_sb, rc, Mult
                    )

                for c in range(4):
                    deferred.append((3 + 2 * c, lambda c=c: rchunk(c)))
                deferred.append((11, fmult))
                if it == 3:
                    # pr0 fully normalized soon: its output projection
                    # columns become available side work
                    for et in range(ET):
                        for sc in range(2):
                            side.append(lambda et=et, sc=sc: unit_p3(et, sc))
            for et in range(ET):
                for sc in range(2, 4):
                    side.append(lambda et=et, sc=sc: unit_p3(et, sc))
            # a few drain units go first so the last iteration's deferred
            # normalize doesn't head-block their PSUM evacuations on DVE
            pump(6)
            flush_deferred(ST)
            pump(len(side))

    _split_multi_waits(nc)
    return nc


def _prepare(x, Wq, bq, Wk, bk, Wv, bv, Wo, bo):
    """Host-side sharding: build per-core input maps (bf16 operands)."""
    import ml_dtypes

    bf16 = ml_dtypes.bfloat16
    x = np.asarray(x, dtype=np.float32)
    Wq = np.asarray(Wq, dtype=np.float32)
    bq = np.asarray(bq, dtype=np.float32)
    Wk = np.asarray(Wk, dtype=np.float32)
    bk = np.asarray(bk, dtype=np.float32)
    Wv = np.asarray(Wv, dtype=np.float32)
    bv = np.asarray(bv, dtype=np.float32)
    Wo = np.asarray(Wo, dtype=np.float32)

    isd = np.float32(1.0 / np.sqrt(D))

    # pre-tile everything into the partition-major SBUF images the kernel
    # DMAs verbatim: contiguous per-partition rows = minimal descriptors
    def tile_w(w):  # [E, M] -> [128, ET, M]
        m = w.shape[1]
        return np.ascontiguousarray(
            w.reshape(ET, 128, m).transpose(1, 0, 2)
        ).astype(bf16)

    xds = [
        np.ascontiguousarray(
            x[b].T.reshape(ET, 128, SC, 512).transpose(1, 2, 0, 3)
        ).astype(bf16)
        for b in range(B)
    ]
    Wq_s = tile_w(Wq * isd)
    Wk_s = tile_w(Wk)
    Wv_s = tile_w(Wv)
    Wo_t = np.ascontiguousarray(
        Wo.reshape(G, R, 128, E).transpose(2, 0, 1, 3)
    ).astype(bf16)  # [128, G, R, E]
    in_maps = []
    for core in range(8):
        b, g = divmod(core, G)
        in_maps.append({
            "xd": xds[b],
            "wq": np.ascontiguousarray(
                Wq_s[:, :, g * R * D : (g + 1) * R * D]
            ),
            "wk": np.ascontiguousarray(Wk_s[:, :, g * D : (g + 1) * D]),
            "wv": np.ascontiguousarray(Wv_s[:, :, g * D : (g + 1) * D]),
            "wo": np.ascontiguousarray(Wo_t[:, g]),
            "bqv": bq[g * R * D : (g + 1) * R * D] * isd,
            "bkv": bk[g * D : (g + 1) * D],
            "bvv": bv[g * D : (g + 1) * D],
        })
    return in_maps


def _gather(results, bo):
    bo = np.asarray(bo, dtype=np.float32)
    out = np.empty((B, S, E), dtype=np.float32)
    for b in range(B):
        acc = results[b * G]["ot"].astype(np.float32)
        for g in range(1, G):
            acc += results[b * G + g]["ot"].astype(np.float32)
        out[b] = acc.T + bo
    return out


def kernel(x, Wq, bq, Wk, bk, Wv, bv, Wo, bo):
    from concourse.bass_utils import run_bass_kernel_spmd

    if "nc" not in _cache:
        _cache["nc"] = _build_program()
    nc = _cache["nc"]
    in_maps = _prepare(x, Wq, bq, Wk, bk, Wv, bv, Wo, bo)
    res = run_bass_kernel_spmd(nc, in_maps, core_ids=list(range(8)))
    return _gather(res.results, bo)
